# revision 1
# baseline (speedup 1.0000x reference)
"""Circular-convolution helper kernel for Trainium2 (8 NeuronCores).

Math: out[i] = sum_b sum_t x1[b,(i-t)%D] * x2[b,t]
            = sum_j G[j, (i-j)%D]   where G = x1^T @ x2  ([D, D], K=B contraction)

Sharding: G's rows are split across the 8 cores (core c owns rows
[128c, 128c+128)).  Per core:
  1. load xin = [x1c | x2] as one [128, 128+D] tensor, split into a 2x2
     row/column grid across the two HWDGE queues so the first G chunk can
     start as soon as the first column block lands
  2. A = x1c^T @ x2 into PSUM (K=128 fp16 matmul, 4 column chunks into
     separate PSUM banks so matmuls never serialize against the casts)
  3. PSUM -> SBUF casts (fp32 -> fp16, Scalar/Vector alternating) into a
     [128, 128+D] staging tile laid out as [A[:, 896:1024] | A] so the DRAM
     scatter is ONE contiguous region per row
  4. scatter to gd[128, 1152]: flat[1152 m + p] = staged row, row-split x4
     across both HWDGE queues
  5. diagonal read H[m, i] = A[m, (i-m) % D] = gd_flat[128 + 1151 m + i],
     chained per row block so reads stream behind the writes
  6. ones-matmul partition collapse (fp16, K-split so the first half starts
     after the first two diagonal reads): part[i] = sum_m H[m, i]
Host rotates each core's partial by 128c and sums.

Everything on the PE runs in fp16 (single-pass streaming, 10-bit mantissa
— same effective precision as tf32-style fp32r but half the DMA bytes).
PSUM accumulation stays fp32.
"""

import numpy as np

B = 128
DIM = 1024
NCORES = 8
CHUNK = DIM // NCORES  # 128 rows of G per core
NHALF = 512
NCHUNKS = 4
CW = DIM // NCHUNKS  # 256
XW = DIM + CHUNK  # packed input width: x1c | x2
XSPLIT = CHUNK + NHALF  # first column block: x1c + x2[:, 0:512]
AW = CHUNK + DIM  # staging width: wrap tail | A

USE_FP16 = True


_cached = {}


def _build():
    key = ("nc", USE_FP16)
    if key in _cached:
        return _cached[key]

    import concourse.bass as bass
    import concourse.mybir as mybir
    from concourse import bacc
    from concourse.tile import TileContext

    f32 = mybir.dt.float32
    dt_mm = mybir.dt.float16 if USE_FP16 else f32

    nc = bacc.Bacc("TRN2", target_bir_lowering=False, debug=False)

    xin = nc.dram_tensor("xin", [B, XW], dt_mm, kind="ExternalInput")
    out = nc.dram_tensor("out", [1, DIM], f32, kind="ExternalOutput")
    gd = nc.dram_tensor("gd", [CHUNK, AW], dt_mm, kind="Internal")

    with TileContext(nc) as tc:
        with (
            tc.tile_pool(name="sb", bufs=1) as sb,
            tc.tile_pool(name="ps", bufs=1, space="PSUM") as ps,
        ):
            xt = sb.tile([B, XW], dt_mm)
            xin_ap = xin.ap()
            nc.sync.dma_start(xt[0:64, 0:XSPLIT], xin_ap[0:64, 0:XSPLIT])
            nc.scalar.dma_start(xt[64:B, 0:XSPLIT], xin_ap[64:B, 0:XSPLIT])
            nc.sync.dma_start(xt[0:64, XSPLIT:XW], xin_ap[0:64, XSPLIT:XW])
            nc.scalar.dma_start(xt[64:B, XSPLIT:XW], xin_ap[64:B, XSPLIT:XW])
            x1_mm = xt[:, 0:CHUNK]

            gs = [
                ps.tile([CHUNK, CW], f32, name=f"g{i}", tag=f"g{i}")
                for i in range(NCHUNKS)
            ]
            a = sb.tile([CHUNK, AW], dt_mm)
            ht = sb.tile([CHUNK, DIM], dt_mm)
            ones = sb.tile([CHUNK, 1], dt_mm)
            nc.vector.memset(ones[:], 1.0)
            os_ = [
                ps.tile([1, CW], f32, name=f"o{i}", tag=f"o{i}")
                for i in range(NCHUNKS)
            ]
            gd_ap = gd.ap()

            # A chunks; staging layout: a[:, 0:128] = A[:, 896:1024] (wrap
            # tail), a[:, 128:1152] = A[:, 0:1024]
            # G matmul K-split over batch halves: the first accumulation
            # pass starts as soon as the first load row-half lands
            order = [0, 1, 2, 3]
            for ch in order:
                lo, hi = ch * CW, (ch + 1) * CW
                nc.tensor.matmul(
                    gs[ch][:], x1_mm[0:64, :], xt[0:64, CHUNK + lo : CHUNK + hi],
                    start=True, stop=False,
                )
            for i, ch in enumerate(order):
                lo, hi = ch * CW, (ch + 1) * CW
                nc.tensor.matmul(
                    gs[ch][:], x1_mm[64:B, :], xt[64:B, CHUNK + lo : CHUNK + hi],
                    start=False, stop=True,
                )
                # alternate cast engine so the cast chain is half as long
                if i % 2 == 0:
                    nc.scalar.copy(a[:, CHUNK + lo : CHUNK + hi], gs[ch][:])
                else:
                    nc.vector.tensor_copy(a[:, CHUNK + lo : CHUNK + hi], gs[ch][:])
                if ch == 3:
                    # wrap tail: A cols [896, 1024) = chunk 3 cols [128, 256)
                    nc.vector.tensor_copy(a[:, 0:CHUNK], gs[ch][:, CHUNK:CW])

            # scatter + diagonal read, chained in 4 row blocks so reads
            # stream right behind the writes.
            # H[m, i] = gd_flat[128 + 1151 m + i]
            for q in range(4):
                r0, r1 = q * 32, (q + 1) * 32
                w_eng = nc.sync if q % 2 == 0 else nc.scalar
                w_eng.dma_start(gd_ap[r0:r1, :], a[r0:r1, :])
            for q in range(4):
                r0, r1 = q * 32, (q + 1) * 32
                diag = bass.AP(
                    gd, CHUNK + r0 * (AW - 1), [[AW - 1, 32], [1, DIM]]
                )
                r_eng = nc.sync if q % 2 == 0 else nc.scalar
                r_eng.dma_start(ht[r0:r1, :], diag)

            # ones-matmul split over K (row halves) so the first half runs
            # as soon as the first diagonal read lands
            ot = sb.tile([1, DIM], f32)
            for ch in range(NCHUNKS):
                lo, hi = ch * CW, (ch + 1) * CW
                nc.tensor.matmul(
                    os_[ch][:], ones[0:64], ht[0:64, lo:hi],
                    start=True, stop=False,
                )
            for ch in range(NCHUNKS):
                lo, hi = ch * CW, (ch + 1) * CW
                nc.tensor.matmul(
                    os_[ch][:], ones[64:CHUNK], ht[64:CHUNK, lo:hi],
                    start=False, stop=True,
                )
                if ch % 2 == 0:
                    nc.scalar.copy(ot[:, lo:hi], os_[ch][:])
                else:
                    nc.vector.tensor_copy(ot[:, lo:hi], os_[ch][:])
            nc.sync.dma_start(out.ap(), ot[:])

    nc.compile()
    _cached[key] = nc
    return nc


def _in_maps(input1, input2):
    dt_in = np.float16 if USE_FP16 else np.float32
    x1 = np.asarray(input1, dtype=np.float32)
    x2 = np.asarray(input2, dtype=np.float32)
    maps = []
    for c in range(NCORES):
        xin = np.empty((B, XW), dt_in)
        xin[:, 0:CHUNK] = x1[:, c * CHUNK : (c + 1) * CHUNK]
        xin[:, CHUNK:XW] = x2
        maps.append({"xin": np.ascontiguousarray(xin)})
    return maps


def _combine(results):
    total = np.zeros(DIM, np.float64)
    for c in range(NCORES):
        total += np.roll(results[c]["out"][0].astype(np.float64), CHUNK * c)
    return total.astype(np.float32).reshape(1, 1, DIM)


def _run(input1, input2, **kwargs):
    from concourse import bass_utils

    nc = _build()
    res = bass_utils.run_bass_kernel_spmd(
        nc, _in_maps(input1, input2), core_ids=list(range(NCORES)), **kwargs
    )
    return res


def kernel(input1, input2):
    res = _run(input1, input2)
    return _combine(res.results)



# revision 2
# speedup vs baseline: 1.2072x; 1.2072x over previous
"""Circular-convolution helper kernel for Trainium2 (8 NeuronCores).

Math: out[i] = sum_b sum_t x1[b,(i-t)%D] * x2[b,t]
            = anti-diagonal sums of G = x1^T @ x2  ([D, D], K=B contraction)

Block scheme (D = 1024 = 8 blocks of 128):
  G block (J, T) (each [128, 128]) contributes its anti-diagonal sums to
  out chunks (J+T)%8 and (J+T+1)%8.  Summing blocks with equal (J+T)%8=S
  first:
      M_S = sum_J G[J, (S-J)%8] = sum_J x1_J^T @ x2_{(S-J)%8}
  then d_S[k] = sum_{a+b=k} M_S[a, b]  (k in [0, 255))
  and  out[128 S + r] = d_S[r] + d_{(S-1)%8}[128 + r].

Sharding: core S computes M_S ([128, 128], K = 8*B = 1024 via 8
accumulating PE matmuls) and d_S.  The per-core block rotation of x2 is
done host-side (packing), so the single SPMD program is core-independent.

The anti-diagonal sum never touches DRAM: a gpsimd local_scatter places
row a of M_S at offset a in a zero-filled [128, 256] tile (indices from
a gpsimd iota), then the columns are summed (ones-matmul on the PE, or a
gpsimd partition_all_reduce).  The kernel has exactly 3 DMAs: 2 input
loads (sync + scalar engine queues in parallel) and 1 result store.

Host combines: out[128 S + r] = d_S[r] + d_{(S-1)%8}[128 + r].
"""

import numpy as np

B = 128
DIM = 1024
NCORES = 8
CH = DIM // NCORES  # 128 = block width
XW = 2 * DIM  # packed input: x1 | x2rot

# "pe": ones-matmul partition reduction; "gp": gpsimd partition_all_reduce
REDUCE_MODE = "pe"

_cached = {}


def _build():
    key = ("nc", REDUCE_MODE)
    if key in _cached:
        return _cached[key]

    import concourse.bass as bass  # noqa: F401
    import concourse.mybir as mybir
    from concourse import bacc, bass_isa
    from concourse.tile import TileContext

    f32 = mybir.dt.float32
    f16 = mybir.dt.float16
    i16 = mybir.dt.int16

    nc = bacc.Bacc("TRN2", target_bir_lowering=False, debug=False)

    xin = nc.dram_tensor("xin", [B, XW], f16, kind="ExternalInput")
    out = nc.dram_tensor("out", [1, 2 * CH], f32, kind="ExternalOutput")

    with TileContext(nc) as tc:
        with (
            tc.tile_pool(name="sb", bufs=1) as sb,
            tc.tile_pool(name="ps", bufs=1, space="PSUM") as ps,
        ):
            xt = sb.tile([B, XW], f16)
            xin_ap = xin.ap()
            nc.sync.dma_start(xt[:, 0:DIM], xin_ap[:, 0:DIM])
            nc.scalar.dma_start(xt[:, DIM:XW], xin_ap[:, DIM:XW])

            # sidx[a, c] = a + c, the shifted-placement scatter indices
            sidx = sb.tile([CH, CH], i16)
            nc.gpsimd.iota(sidx[:], pattern=[[1, CH]], channel_multiplier=1)

            # M_S = sum_J x1_J^T @ x2rot_J   (K-accumulation over 8 blocks)
            mp = ps.tile([CH, CH], f32)
            for j in range(8):
                lo = j * CH
                nc.tensor.matmul(
                    mp[:],
                    xt[:, lo : lo + CH],
                    xt[:, DIM + lo : DIM + lo + CH],
                    start=(j == 0),
                    stop=(j == 7),
                )

            msb = sb.tile([CH, CH], f16)
            nc.vector.tensor_copy(msb[:], mp[:])

            # tsb[a, a + c] = M[a, c], zeros elsewhere
            tsb = sb.tile([CH, 2 * CH], f16)
            nc.gpsimd.local_scatter(
                tsb[:], msb[:], sidx[:], channels=CH, num_elems=2 * CH, num_idxs=CH
            )

            if REDUCE_MODE == "gp":
                rsb = sb.tile([CH, 2 * CH], f32)
                nc.gpsimd.partition_all_reduce(
                    rsb[:], tsb[:], channels=CH, reduce_op=bass_isa.ReduceOp.add
                )
                nc.sync.dma_start(out.ap(), rsb[0:1, :])
            else:
                ones = sb.tile([CH, 1], f16)
                nc.vector.memset(ones[:], 1.0)
                op_ = ps.tile([1, 2 * CH], f32)
                nc.tensor.matmul(op_[:], ones[:], tsb[:], start=True, stop=True)
                ot = sb.tile([1, 2 * CH], f32)
                nc.vector.tensor_copy(ot[:], op_[:])
                nc.sync.dma_start(out.ap(), ot[:])

    nc.compile()
    _cached[key] = nc
    return nc


def _in_maps(input1, input2):
    x1 = np.asarray(input1, dtype=np.float16)
    x2 = np.asarray(input2, dtype=np.float16)
    maps = []
    for s in range(NCORES):
        xin = np.empty((B, XW), np.float16)
        xin[:, 0:DIM] = x1
        for j in range(8):
            t = (s - j) % 8
            xin[:, DIM + j * CH : DIM + (j + 1) * CH] = x2[:, t * CH : (t + 1) * CH]
        maps.append({"xin": np.ascontiguousarray(xin)})
    return maps


def _combine(results):
    d = np.stack(
        [results[s]["out"][0].astype(np.float64) for s in range(NCORES)]
    )  # [8, 256]
    out = np.empty(DIM, np.float64)
    for s in range(NCORES):
        out[s * CH : (s + 1) * CH] = d[s, 0:CH] + d[(s - 1) % 8, CH : 2 * CH]
    return out.astype(np.float32).reshape(1, 1, DIM)


def _run(input1, input2, **kwargs):
    from concourse import bass_utils

    nc = _build()
    res = bass_utils.run_bass_kernel_spmd(
        nc, _in_maps(input1, input2), core_ids=list(range(NCORES)), **kwargs
    )
    return res


def kernel(input1, input2):
    res = _run(input1, input2)
    return _combine(res.results)


# revision 4
# speedup vs baseline: 1.3292x; 1.1011x over previous
"""Circular-convolution helper kernel for Trainium2 (8 NeuronCores).

Math: out[i] = sum_b sum_t x1[b,(i-t)%D] * x2[b,t]
            = anti-diagonal sums of G = x1^T @ x2  ([D, D], K=B contraction)

Block scheme (D = 1024 = 8 blocks of 128):
  G block (J, T) (each [128, 128]) contributes its anti-diagonal sums to
  out chunks (J+T)%8 and (J+T+1)%8.  Summing blocks with equal (J+T)%8=S
  first:
      M_S = sum_J G[J, (S-J)%8] = sum_J x1_J^T @ x2_{(S-J)%8}
  then d_S[k] = sum_{a+b=k} M_S[a, b]  (k in [0, 255))
  and  out[128 S + r] = d_S[r] + d_{(S-1)%8}[128 + r].

Sharding: core S computes M_S ([128, 128], K = 8*B = 1024 via 8
accumulating PE matmuls) and d_S.  The per-core block rotation of x2 is
done host-side (packing), so the single SPMD program is core-independent.

The anti-diagonal sum never touches DRAM: a gpsimd local_scatter places
row a of M_S at free-offset a in a zero-filled [128, 256] tile (indices
idx[a,c] = a + c shipped from the host), then a ones-matmul on the PE
sums the columns.  3 DMAs total: x1, x2rot (+ tiny index load) in, and
the [1, 256] result out.

Raw bass (no TileContext): 9 hand-managed semaphores with
wait-and-decrement so every semaphore returns to 0 after each NEFF
execution (safe re-execution without a multi-microsecond semaphore-reset
epilogue).  The gpsimd ucode library for local_scatter is loaded
explicitly at kernel start, hidden under the input-DMA latency.

Host combines: out[128 S + r] = d_S[r] + d_{(S-1)%8}[128 + r].
"""

import numpy as np

B = 128
DIM = 1024
NCORES = 8
CH = DIM // NCORES  # 128 = block width
XW = 2 * DIM  # packed input: x1 | x2rot

_cached = {}


def _build():
    key = "nc"
    if key in _cached:
        return _cached[key]

    import concourse.mybir as mybir
    from concourse import bacc, library_config

    f32 = mybir.dt.float32
    f16 = mybir.dt.float16
    i16 = mybir.dt.int16

    nc = bacc.Bacc("TRN2", target_bir_lowering=False, debug=False)

    xin = nc.dram_tensor("xin", [B, XW], f16, kind="ExternalInput")
    sidx_d = nc.dram_tensor("sidx", [CH, CH], i16, kind="ExternalInput")
    out = nc.dram_tensor("out", [1, 2 * CH], f32, kind="ExternalOutput")

    with (
        nc.semaphore("s_x1") as s_x1,
        nc.semaphore("s_x2") as s_x2,
        nc.semaphore("s_si") as s_si,
        nc.semaphore("s_mm") as s_mm,
        nc.semaphore("s_cast") as s_cast,
        nc.semaphore("s_scat") as s_scat,
        nc.semaphore("s_mm2") as s_mm2,
        nc.semaphore("s_ot") as s_ot,
        nc.semaphore("s_out") as s_out,
        nc.sbuf_tensor("xt", [B, XW], f16) as xt,
        nc.sbuf_tensor("sidx_t", [CH, CH], i16) as sidx_t,
        nc.sbuf_tensor("msb", [CH, CH], f16) as msb,
        nc.sbuf_tensor("tsb", [CH, 2 * CH], f16) as tsb,
        nc.sbuf_tensor("ones", [CH, 1], f16) as ones,
        nc.sbuf_tensor("ot", [1, 2 * CH], f32) as ot,
        nc.psum_tensor("mp", [CH, CH], f32) as mp,
        nc.psum_tensor("op", [1, 2 * CH], f32) as op_,
    ):
        # Re-execution safety: zero every kernel semaphore, then barrier so
        # no engine's first wait can observe a stale value from a previous
        # NEFF execution.  (The barrier sems self-reset: ==N then =0.)
        sems = [s_x1, s_x2, s_si, s_mm, s_cast, s_scat, s_mm2, s_ot, s_out]
        nums = sorted(s.num for s in sems)
        assert nums == list(range(nums[0], nums[0] + len(nums)))
        nc.gpsimd.sem_clear(range(nums[0], nums[-1] + 1))
        nc.all_engine_barrier()

        # off-critical-path setup: the gpsimd ucode library load (~2.3us,
        # blocks only gpsimd) hides under the input-DMA latency
        nc.gpsimd.load_library(library_config.local_scatter)
        nc.vector.memset(ones[:], 1.0)

        xin_ap = xin.ap()
        nc.sync.dma_start(xt[:, 0:DIM], xin_ap[:, 0:DIM]).then_inc(s_x1, 16)
        nc.scalar.dma_start(xt[:, DIM:XW], xin_ap[:, DIM:XW]).then_inc(s_x2, 16)
        nc.scalar.dma_start(sidx_t[:], sidx_d.ap()).then_inc(s_si, 16)

        # M_S = sum_J x1_J^T @ x2rot_J   (K-accumulation over 8 blocks)
        nc.tensor.wait_ge(s_x1, 16)
        nc.tensor.wait_ge(s_x2, 16)
        for j in range(8):
            lo = j * CH
            last = j == 7
            mm = nc.tensor.matmul(
                mp[:],
                xt[:, lo : lo + CH],
                xt[:, DIM + lo : DIM + lo + CH],
                start=(j == 0),
                stop=last,
            )
            if last:
                mm.then_inc(s_mm, 1)

        # PSUM f32 -> SBUF f16
        nc.vector.wait_ge(s_mm, 1)
        nc.vector.tensor_copy(msb[:], mp[:]).then_inc(s_cast, 1)

        # tsb[a, a + c] = M[a, c], zeros elsewhere
        nc.gpsimd.wait_ge(s_cast, 1)
        nc.gpsimd.wait_ge(s_si, 16)
        nc.gpsimd.local_scatter(
            tsb[:], msb[:], sidx_t[:], channels=CH, num_elems=2 * CH, num_idxs=CH
        ).then_inc(s_scat, 1)

        # d_S = column sums of tsb
        nc.tensor.wait_ge(s_scat, 1)
        nc.tensor.matmul(op_[:], ones[:], tsb[:], start=True, stop=True).then_inc(
            s_mm2, 1
        )
        nc.vector.wait_ge(s_mm2, 1)
        nc.vector.tensor_copy(ot[:], op_[:]).then_inc(s_ot, 1)

        nc.sync.wait_ge(s_ot, 1)
        nc.sync.dma_start(out.ap(), ot[:]).then_inc(s_out, 16)
        nc.sync.wait_ge(s_out, 16)

    nc.compile()
    _cached[key] = nc
    return nc


def _sidx():
    a = np.arange(CH, dtype=np.int16)[:, None]
    c = np.arange(CH, dtype=np.int16)[None, :]
    return np.ascontiguousarray(a + c)


def _in_maps(input1, input2):
    x1 = np.asarray(input1, dtype=np.float16)
    x2 = np.asarray(input2, dtype=np.float16)
    sidx = _sidx()
    maps = []
    for s in range(NCORES):
        xin = np.empty((B, XW), np.float16)
        xin[:, 0:DIM] = x1
        for j in range(8):
            t = (s - j) % 8
            xin[:, DIM + j * CH : DIM + (j + 1) * CH] = x2[:, t * CH : (t + 1) * CH]
        maps.append({"xin": np.ascontiguousarray(xin), "sidx": sidx})
    return maps


def _combine(results):
    d = np.stack(
        [results[s]["out"][0].astype(np.float64) for s in range(NCORES)]
    )  # [8, 256]
    out = np.empty(DIM, np.float64)
    for s in range(NCORES):
        out[s * CH : (s + 1) * CH] = d[s, 0:CH] + d[(s - 1) % 8, CH : 2 * CH]
    return out.astype(np.float32).reshape(1, 1, DIM)


def _run(input1, input2, **kwargs):
    from concourse import bass_utils

    nc = _build()
    res = bass_utils.run_bass_kernel_spmd(
        nc, _in_maps(input1, input2), core_ids=list(range(NCORES)), **kwargs
    )
    return res


def kernel(input1, input2):
    res = _run(input1, input2)
    return _combine(res.results)
